# revision 1
# baseline (speedup 1.0000x reference)
"""Gated GQA self-attention with KV cache, tensor-parallel over heads on 8
Trainium2 NeuronCores.

Reference computation (fp32):
    q = rms_norm((x @ w_q.T).reshape(B,L,H,HD))      # per-head rms over HD
    k = rms_norm((x @ w_k.T).reshape(B,L,HKV,HD))
    v = (x @ w_v.T).reshape(B,L,HKV,HD)
    k_t/v_t = concat(cache, new) over seq -> [B,HKV,S,HD]
    o = softmax(q @ k_t.T / sqrt(HD)) @ v_t          # full (non-causal)
    o *= sigmoid(x[..., :16] @ w_gate.T)             # per-head gate
    y = o.reshape(B,L,D) @ w_out.T

Sharding: core c owns q heads {2c, 2c+1} and kv group g=c//2 (GQA groups
stay intact).  Each core computes its heads' attention plus the partial
out-projection y_c = o_c @ w_out[:, cols_c].T; the host sums the 8
partials (replaces the all-reduce).

Device-side layout: everything is computed feature-on-partition
("transposed"), so scores come out [s, l] and the P matrix never needs a
transpose for the p@v matmul.  The host pre-transposes x and the weights
so the device never transposes activations either.  Matmuls run in
float32r (fp32 stored with tf32-like operand rounding, ~5e-4 relative;
1 cycle/row when the moving free dim >= 256 vs 4 for plain fp32).

Softmax runs without max-subtraction (scores here are ~N(0,1); exp
cannot overflow).  The softmax denominator is a ones-matmul partition
sum; per-column factors (q-rms scale, gate/denominator) are applied as
rank-1 PE broadcasts; the k-rms scale rides the exp activation's
per-partition scale operand.  1-partition rows are reshaped to [128, n]
via small internal-DRAM bounce DMAs so reciprocals never run on a
single DVE lane.
"""

from contextlib import ExitStack

import numpy as np

import concourse.bass as bass
import concourse.tile as tile
from concourse import bacc, mybir
from concourse.bass_utils import run_bass_kernel_spmd

F32R = mybir.dt.float32r
F32 = mybir.dt.float32
AF = mybir.ActivationFunctionType

B, L, D = 2, 1024, 2048
H, HKV, HD = 16, 4, 128
CACHE = 1024
BL = B * L                  # 2048
S = CACHE + L               # 2048
NCORES = 8
QH = H // NCORES            # 2 q heads per core
JC = QH * HD                # 256 out-proj contraction cols per core
EPS = 1e-6

_CACHED_NC = None


def _build_core_program():
    """One SPMD program; per-core differences are input data only."""
    nc = bacc.Bacc("TRN2", target_bir_lowering=False, debug=False)

    xt = nc.dram_tensor("xt", [D, BL], F32R, kind="ExternalInput").ap()
    wqkv = nc.dram_tensor("wqkv", [D, 4 * HD], F32R, kind="ExternalInput").ap()
    wo = nc.dram_tensor("wo", [JC, D], F32R, kind="ExternalInput").ap()
    wg = nc.dram_tensor("wg", [H, QH], F32R, kind="ExternalInput").ap()
    ckt = nc.dram_tensor("ckt", [B, HD, CACHE], F32R, kind="ExternalInput").ap()
    cv = nc.dram_tensor("cv", [B, CACHE, HD], F32R, kind="ExternalInput").ap()
    # [:, :128] identity for PE transposes, [:, 128] all-ones column
    consts_in = nc.dram_tensor("consts", [128, 129], F32R, kind="ExternalInput").ap()
    onesr_in = nc.dram_tensor("onesr", [1, 128], F32R, kind="ExternalInput").ap()
    y = nc.dram_tensor("y", [BL, D], F32, kind="ExternalOutput").ap()

    # internal-DRAM bounce buffers for row<->column reshapes
    qscr = nc.dram_tensor("qscr", [QH, 16, 128], F32R).ap()
    kscr = nc.dram_tensor("kscr", [16, 128], F32).ap()
    dscr = nc.dram_tensor("dscr", [8, 4, 128], F32).ap()

    NLP = BL // 256          # 8 column chunks for the x stream
    ND = D // 128            # 16 contraction chunks for the projections
    NS = S // 128            # 16 s chunks per batch
    NSC = CACHE // 128       # 8 cached s chunks

    with tile.TileContext(nc) as tc, ExitStack() as ctx:
        singles = ctx.enter_context(tc.tile_pool(name="singles", bufs=1))
        xtp = ctx.enter_context(tc.tile_pool(name="xtp", bufs=2))
        # rotating pool of [128, <=512] working tiles: exp chunks, squares,
        # bcast factors, output staging
        work = ctx.enter_context(tc.tile_pool(name="work", bufs=7))
        cachep = ctx.enter_context(tc.tile_pool(name="cachep", bufs=1))
        frp = ctx.enter_context(tc.tile_pool(name="frp", bufs=2))
        colp = ctx.enter_context(tc.tile_pool(name="colp", bufs=2))

        psA = ctx.enter_context(tc.tile_pool(name="psA", bufs=3, space="PSUM"))
        psO = ctx.enter_context(tc.tile_pool(name="psO", bufs=2, space="PSUM"))
        psD = ctx.enter_context(tc.tile_pool(name="psD", bufs=1, space="PSUM"))
        psF = ctx.enter_context(tc.tile_pool(name="psF", bufs=2, space="PSUM"))

        lowp = nc.allow_low_precision(reason="float32r rounding is intended")
        ctx.enter_context(lowp)

        consts = singles.tile([128, 129], F32R)
        nc.scalar.dma_start(out=consts, in_=consts_in)
        ident = consts[:, 0:128]
        ones_col = consts[:, 128:129]
        ones_row = singles.tile([1, 128], F32R)
        nc.scalar.dma_start(out=ones_row, in_=onesr_in)

        bias_q = singles.tile([1, 1], F32)
        nc.vector.memset(bias_q, HD * EPS)
        bias_k = singles.tile([1, 1], F32)
        nc.vector.memset(bias_k, EPS)

        wg_sb = singles.tile([H, QH], F32R)
        nc.scalar.dma_start(out=wg_sb, in_=wg)
        # split the weight load so the first projection matmuls start after
        # ~1/4 of it has landed
        wqkv_sb = singles.tile([128, ND, 4 * HD], F32R)
        wqkv_r = wqkv.rearrange("(k p) j -> p k j", p=128)
        for kq in range(4):
            nc.sync.dma_start(
                out=wqkv_sb[:, kq * 4 : kq * 4 + 4, :],
                in_=wqkv_r[:, kq * 4 : kq * 4 + 4, :],
            )
        wo_sb = singles.tile([128, QH, D], F32R)

        # persistent activations, feature-on-partition
        qkvt = singles.tile([128, 4, BL], F32R)       # jc: qh0, qh1, k, v
        otg = singles.tile([128, B, QH, 2, 512], F32R)  # raw attention out
        gcol = singles.tile([128, 16, QH], F32)       # gates, l-on-partition
        fcols = singles.tile([128, B, QH, 2, 4], F32)   # gate/den columns
        qs = [
            singles.tile([1, BL], F32R, tag=f"qs{i}", name=f"qs{i}")
            for i in range(QH)
        ]  # q rms scale rows (sqrt then reciprocal via bounce)
        kcol = singles.tile([128, 16], F32)          # k rms scale columns

        ksr = singles.tile([1, BL], F32)             # k sqrt staging row
        xg = singles.tile([H, BL], F32R)              # x[..., :16] for gates
        cache_tiles = {}

        def emit_prefetch():
            """Non-critical loads, queued after the first x tile so they
            don't delay the first projection matmul."""
            nc.scalar.dma_start(
                out=wo_sb, in_=wo.rearrange("(h p) m -> p h m", p=128)
            )
            nc.scalar.dma_start(out=xg, in_=xt[0:H, :])
            for b in range(B):
                ck_sb = cachep.tile(
                    [128, CACHE], F32R, tag=f"ck{b}", name=f"ck{b}"
                )
                nc.scalar.dma_start(out=ck_sb, in_=ckt[b])
                cv_sb = cachep.tile(
                    [128, NSC, HD], F32R, tag=f"cv{b}", name=f"cv{b}"
                )
                nc.scalar.dma_start(
                    out=cv_sb, in_=cv[b].rearrange("(i p) d -> p i d", p=128)
                )
                cache_tiles[b] = (ck_sb, cv_sb)

        # ---- phase 1: projections -------------------------------------
        def finish_half(half):
            """Reciprocal of the rms rows via DRAM bounce (1-lane DVE rows
            are ~6us each) + qT column normalize — per half so the second
            half overlaps remaining projection work."""
            rs = slice(half * 8, half * 8 + 8)
            row_sl = slice(half * 1024, half * 1024 + 1024)
            nc.scalar.dma_start(out=kscr[rs], in_=ksr[:, row_sl])
            nc.scalar.dma_start(
                out=kcol[:, half * 8 : half * 8 + 8],
                in_=kscr[rs].rearrange("c p -> p c"),
            )
            nc.vector.reciprocal(
                kcol[:, half * 8 : half * 8 + 8], kcol[:, half * 8 : half * 8 + 8]
            )
            for h in range(QH):
                nc.scalar.dma_start(out=qscr[h, rs], in_=qs[h][:, row_sl])
                qc = colp.tile([128, 8], F32R, tag="qcol", name=f"qc{h}_{half}")
                nc.scalar.dma_start(out=qc, in_=qscr[h, rs].rearrange("c p -> p c"))
                nc.vector.reciprocal(qc, qc)
                nc.scalar.dma_start(out=qscr[h, rs].rearrange("c p -> p c"), in_=qc)
                nc.scalar.dma_start(
                    out=qs[h][:, row_sl], in_=qscr[h, rs].flatten().unsqueeze(0)
                )
                for lc in range(2):
                    sl = slice(half * 1024 + lc * 512, half * 1024 + lc * 512 + 512)
                    bc = psF.tile([128, 512], F32, tag="psF", name="bc")
                    nc.tensor.matmul(
                        bc, ones_row, qs[h][:, sl], start=True, stop=True
                    )
                    nc.vector.tensor_mul(qkvt[:, h, sl], qkvt[:, h, sl], bc)

        xt_r = xt.rearrange("(k p) l -> p k l", p=128)
        for lc in range(NLP):
            sl = slice(lc * 256, lc * 256 + 256)
            xtile = xtp.tile([128, ND, 256], F32R, tag="xt")
            for kq in range(4):
                nc.sync.dma_start(
                    out=xtile[:, kq * 4 : kq * 4 + 4, :],
                    in_=xt_r[:, kq * 4 : kq * 4 + 4, sl],
                )
            if lc == 0:
                emit_prefetch()
            for jc in (3, 2, 0, 1):  # v and k first: unblocks attention prep
                pp = psA.tile([128, 256], F32, tag="psA")
                for kk in range(ND):
                    nc.tensor.matmul(
                        pp,
                        wqkv_sb[:, kk, jc * 128 : jc * 128 + 128],
                        xtile[:, kk, :],
                        start=(kk == 0),
                        stop=(kk == ND - 1),
                    )
                nc.vector.tensor_copy(qkvt[:, jc, sl], pp)
                if jc < 3:  # q0, q1, k need sum over HD of the square
                    sq = work.tile([128, 256], F32R, tag="work", name=f"sq{lc}_{jc}")
                    nc.vector.tensor_mul(sq, qkvt[:, jc, sl], qkvt[:, jc, sl])
                    ssq = psD.tile([1, 256], F32, tag="psD")
                    nc.tensor.matmul(ssq, ones_col, sq, start=True, stop=True)
                    # q: sqrt(ssq + HD*eps) so the reciprocal also folds in
                    # the 1/sqrt(HD) score scale; k: sqrt(ssq/HD + eps).
                    row = qs[jc] if jc < QH else ksr
                    scale, bias = (1.0, bias_q) if jc < QH else (1.0 / HD, bias_k)
                    nc.scalar.activation(
                        row[:, sl], ssq, AF.Sqrt, bias=bias[:], scale=scale
                    )
            if lc == 3:
                finish_half(0)
        finish_half(1)
        # gates in column form: [l-part, chunk, head]
        gps = psF.tile([128, 16, QH], F32, tag="psF", name="gps")
        for c in range(16):
            nc.tensor.matmul(
                gps[:, c, :],
                xg[:, c * 128 : c * 128 + 128],
                wg_sb,
                start=True,
                stop=True,
            )
        nc.scalar.activation(gcol, gps, AF.Sigmoid)

        # ---- phase 2: attention ---------------------------------------
        for b in range(B):
            boff = b * L
            ck_sb, cv_sb = cache_tiles[b]
            vnew = cachep.tile([128, NSC, HD], F32R, tag=f"vnew{b}", name=f"vn{b}")
            for i in range(NSC):
                tp = psF.tile([128, 128], F32R, tag="psF", name="tp")
                nc.tensor.transpose(
                    tp, qkvt[:, 3, boff + i * 128 : boff + i * 128 + 128], ident
                )
                nc.vector.tensor_copy(vnew[:, i, :], tp)

            for h in range(QH):
                for lc2 in range(2):
                    it = (b * QH + h) * 2 + lc2
                    off = boff + lc2 * 512
                    qsl = qkvt[:, h, off : off + 512]
                    den = psD.tile([1, 512], F32, tag="psD")
                    ot = psO.tile([128, 512], F32)
                    for sc in range(NS):
                        if sc < NSC:
                            kT = ck_sb[:, sc * 128 : sc * 128 + 128]
                            vx = cv_sb[:, sc, :]
                            kscale = 1.0
                        else:
                            j = boff + (sc - NSC) * 128
                            kT = qkvt[:, 2, j : j + 128]
                            vx = vnew[:, sc - NSC, :]
                            cglob = (sc - NSC) + 8 * b
                            kscale = kcol[:, cglob : cglob + 1]
                        sp = psA.tile([128, 512], F32, tag="psA")
                        nc.tensor.matmul(sp, kT, qsl, start=True, stop=True)
                        ex = work.tile([128, 512], F32R, tag="work", name=f"ex{sc}")
                        nc.scalar.activation(ex, sp, AF.Exp, scale=kscale)
                        nc.tensor.matmul(
                            den, ones_col, ex,
                            start=(sc == 0), stop=(sc == NS - 1),
                        )
                        nc.tensor.matmul(
                            ot, vx, ex,
                            start=(sc == 0), stop=(sc == NS - 1),
                        )
                    # evacuate raw attention out immediately so the PSUM
                    # accumulator recycles without waiting on the factor
                    # chain; gate/den applied in phase 3 as a per-partition
                    # scale
                    nc.vector.tensor_copy(otg[:, b, h, lc2, :], ot)
                    drow = frp.tile([1, 512], F32, tag="drow", name="drow")
                    nc.scalar.copy(drow, den)
                    nc.scalar.dma_start(out=dscr[it], in_=drow)
                    dcol = colp.tile([128, 4], F32, tag="dcol", name="dcol")
                    nc.scalar.dma_start(
                        out=dcol, in_=dscr[it].rearrange("c p -> p c")
                    )
                    nc.vector.reciprocal(dcol, dcol)
                    nc.vector.tensor_mul(
                        fcols[:, b, h, lc2, :],
                        dcol,
                        gcol[:, 8 * b + 4 * lc2 : 8 * b + 4 * lc2 + 4, h],
                    )

        # ---- phase 3: partial out-projection --------------------------
        for b in range(B):
            for lc2 in range(2):
                for li in range(4):
                    row0 = b * L + lc2 * 512 + li * 128
                    for mc in range(4):
                        yps = []
                        for h in range(QH):
                            yp = psA.tile(
                                [128, 512], F32, tag="psA", name=f"yp{h}"
                            )
                            nc.tensor.matmul(
                                yp,
                                otg[:, b, h, lc2, li * 128 : li * 128 + 128],
                                wo_sb[:, h, mc * 512 : mc * 512 + 512],
                                start=True,
                                stop=True,
                            )
                            yps.append(yp)
                        # ysb = f0[l]*yp0 + f1[l]*yp1  (f per-partition);
                        # step 1 on ACT, step 2 on DVE — phase 3 is
                        # evacuation-bound, so split it across engines
                        ysb = work.tile([128, 512], F32, tag="work", name="ysb")
                        nc.scalar.activation(
                            ysb,
                            yps[0],
                            AF.Identity,
                            scale=fcols[:, b, 0, lc2, li : li + 1],
                        )
                        nc.vector.scalar_tensor_tensor(
                            out=ysb,
                            in0=yps[1],
                            scalar=fcols[:, b, 1, lc2, li : li + 1],
                            in1=ysb,
                            op0=mybir.AluOpType.mult,
                            op1=mybir.AluOpType.add,
                        )
                        nc.sync.dma_start(
                            out=y[row0 : row0 + 128, mc * 512 : mc * 512 + 512],
                            in_=ysb,
                        )

    nc.compile()
    return nc


def _get_nc():
    global _CACHED_NC
    if _CACHED_NC is None:
        _CACHED_NC = _build_core_program()
    return _CACHED_NC


def make_in_maps(x, w_q, w_k, w_v, w_out, w_gate, cache_k, cache_v):
    xt = np.ascontiguousarray(x.reshape(BL, D).T)
    consts_np = np.concatenate(
        [np.eye(128, dtype=np.float32), np.ones((128, 1), np.float32)], axis=1
    )
    in_maps = []
    for c in range(NCORES):
        g = c // 2
        wq_c = w_q[c * JC : (c + 1) * JC]                      # [256, D]
        wk_c = w_k[g * HD : (g + 1) * HD]                      # [128, D]
        wv_c = w_v[g * HD : (g + 1) * HD]
        wqkv_c = np.ascontiguousarray(
            np.concatenate([wq_c, wk_c, wv_c], axis=0).T      # [D, 512]
        )
        wo_c = np.ascontiguousarray(w_out[:, c * JC : (c + 1) * JC].T)  # [256, D]
        wg_c = np.ascontiguousarray(w_gate[c * QH : (c + 1) * QH].T)    # [16, 2]
        ckt_c = np.ascontiguousarray(cache_k[:, g].transpose(0, 2, 1))  # [B,HD,CACHE]
        cv_c = np.ascontiguousarray(cache_v[:, g])                      # [B,CACHE,HD]
        in_maps.append(
            {
                "xt": xt,
                "wqkv": wqkv_c,
                "wo": wo_c,
                "wg": wg_c,
                "ckt": ckt_c,
                "cv": cv_c,
                "consts": consts_np,
                "onesr": np.ones((1, 128), np.float32),
            }
        )
    return in_maps


def kernel(x, w_q, w_k, w_v, w_out, w_gate, cache_k, cache_v, _run_kwargs=None):
    in_maps = make_in_maps(x, w_q, w_k, w_v, w_out, w_gate, cache_k, cache_v)
    nc = _get_nc()
    res = run_bass_kernel_spmd(
        nc, in_maps, core_ids=list(range(NCORES)), **(_run_kwargs or {})
    )
    acc = np.zeros((BL, D), dtype=np.float64)
    for c in range(NCORES):
        acc += res.results[c]["y"]
    out = acc.astype(np.float32).reshape(B, L, D)
    if _run_kwargs:
        kernel.last_results = res
    return out



# revision 10
# speedup vs baseline: 1.2928x; 1.2928x over previous
"""Gated GQA self-attention with KV cache, tensor-parallel over heads on 8
Trainium2 NeuronCores.

Reference computation (fp32):
    q = rms_norm((x @ w_q.T).reshape(B,L,H,HD))      # per-head rms over HD
    k = rms_norm((x @ w_k.T).reshape(B,L,HKV,HD))
    v = (x @ w_v.T).reshape(B,L,HKV,HD)
    k_t/v_t = concat(cache, new) over seq -> [B,HKV,S,HD]
    o = softmax(q @ k_t.T / sqrt(HD)) @ v_t          # full (non-causal)
    o *= sigmoid(x[..., :16] @ w_gate.T)             # per-head gate
    y = o.reshape(B,L,D) @ w_out.T

Sharding: core c owns q heads {2c, 2c+1} and kv group g=c//2.  Each core
computes its heads' attention plus the partial out-projection
y_c = o_c @ w_out[:, cols_c].T; the host sums the 8 partials.

v2 design (vs the f32r baseline):
  * bf16 operands everywhere on the matmul paths (x, weights, q/k/v, exp(P),
    V, out-proj); fp32 only in PSUM and the small normalization rows.
    Expected extra error ~5e-3 max-rel, well under the 2e-2 gate; DMA and
    SBUF halve.
  * Softmax denominator off the TensorE: exp chunks are accumulated by a
    small DVE add tree; one ones-matmul per block replaces sixteen.
  * Attention inner loop software-pipelined (scores for chunk pair i+2 are
    issued before P@V of pair i), exp done on [128,2,512] double-chunks to
    amortize the ~250ns ACT fixed cost, so neither PE nor ACT ever blocks
    on the other.
  * Gate/denominator factors are rank-1-broadcast and multiplied into the
    raw attention output once per block (2 cheap ops) instead of per
    out-projection tile (128 expensive ones); the two heads then accumulate
    into a single PSUM in the out-projection.
  * Phase 3 interleaved into attention per (b, lc2) unit, deferred by one
    block so the denominator chain is never on the PE critical path.
  * RMS/denominator reciprocals run on DVE rows directly - no DRAM bounce.
"""

from contextlib import ExitStack

import ml_dtypes
import numpy as np

import concourse.bass as bass  # noqa: F401  (engine types via bacc)
import concourse.tile as tile
from concourse import bacc, mybir
from concourse.bass_utils import run_bass_kernel_spmd

F32R = mybir.dt.float32r
F32 = mybir.dt.float32
BF16 = mybir.dt.bfloat16
AF = mybir.ActivationFunctionType

B, L, D = 2, 1024, 2048
H, HKV, HD = 16, 4, 128
CACHE = 1024
BL = B * L                  # 2048
S = CACHE + L               # 2048
NCORES = 8
QH = H // NCORES            # 2 q heads per core
JC = QH * HD                # 256 out-proj contraction cols per core
EPS = 1e-6
NLP = BL // 512             # 4 l-chunks in phase 1
ND = D // 128               # 16 contraction chunks
NS = S // 128               # 16 s-chunks per batch
NSC = CACHE // 128          # 8 cached s chunks
NP = NS // 2                # 8 s-chunk *pairs* per block

_CACHED_NC = None


def _build_core_program():
    nc = bacc.Bacc("TRN2", target_bir_lowering=False, debug=False)

    xt = nc.dram_tensor("xt", [128, NLP, ND, 512], BF16, kind="ExternalInput").ap()
    wqkv = nc.dram_tensor("wqkv", [128, ND, 4 * HD], BF16, kind="ExternalInput").ap()
    wo = nc.dram_tensor("wo", [128, QH, D], BF16, kind="ExternalInput").ap()
    wg = nc.dram_tensor("wg", [H, QH], BF16, kind="ExternalInput").ap()
    ckt = nc.dram_tensor("ckt", [B, HD, CACHE], BF16, kind="ExternalInput").ap()
    cv = nc.dram_tensor("cv", [B, 128, NSC, HD], BF16, kind="ExternalInput").ap()
    identb_in = nc.dram_tensor("identb", [128, 128], BF16, kind="ExternalInput").ap()
    onesb_in = nc.dram_tensor("onesb", [128, 1], BF16, kind="ExternalInput").ap()
    onesr_in = nc.dram_tensor("onesr", [1, 128], F32R, kind="ExternalInput").ap()
    y = nc.dram_tensor("y", [BL, D], BF16, kind="ExternalOutput").ap()

    with tile.TileContext(nc) as tc, ExitStack() as ctx:
        singles = ctx.enter_context(tc.tile_pool(name="singles", bufs=1))
        xtp = ctx.enter_context(tc.tile_pool(name="xtp", bufs=2))
        sqp = ctx.enter_context(tc.tile_pool(name="sqp", bufs=2))
        exp_ = ctx.enter_context(tc.tile_pool(name="exp", bufs=4))
        accp = ctx.enter_context(tc.tile_pool(name="accp", bufs=2))
        fdnp = ctx.enter_context(tc.tile_pool(name="fdnp", bufs=2))
        ysbp = ctx.enter_context(tc.tile_pool(name="ysbp", bufs=2))

        psBig = ctx.enter_context(tc.tile_pool(name="psBig", bufs=2, space="PSUM"))
        psO = ctx.enter_context(tc.tile_pool(name="psO", bufs=2, space="PSUM"))
        psD = ctx.enter_context(tc.tile_pool(name="psD", bufs=1, space="PSUM"))
        psM = ctx.enter_context(tc.tile_pool(name="psM", bufs=1, space="PSUM"))

        lowp = nc.allow_low_precision(reason="bf16/f32r rounding is intended")
        ctx.enter_context(lowp)

        identb = singles.tile([128, 128], BF16)
        nc.scalar.dma_start(out=identb, in_=identb_in)
        onesb = singles.tile([128, 1], BF16)
        nc.scalar.dma_start(out=onesb, in_=onesb_in)
        onesr = singles.tile([1, 128], F32R)
        nc.scalar.dma_start(out=onesr, in_=onesr_in)
        wg_sb = singles.tile([H, QH], BF16)
        nc.scalar.dma_start(out=wg_sb, in_=wg)

        bias_q = singles.tile([1, 1], F32)
        nc.vector.memset(bias_q, HD * EPS)
        bias_k = singles.tile([1, 1], F32)
        nc.vector.memset(bias_k, EPS)

        wqkv_sb = singles.tile([128, ND, 4 * HD], BF16)
        for kq in range(4):
            nc.sync.dma_start(
                out=wqkv_sb[:, kq * 4 : kq * 4 + 4, :],
                in_=wqkv[:, kq * 4 : kq * 4 + 4, :],
            )
        wo_sb = singles.tile([128, QH, D], BF16)

        # persistent activations, feature-on-partition (bf16)
        qkvt = singles.tile([128, 4, BL], BF16)       # jc: qh0, qh1, k, v
        otg = singles.tile([128, B, QH, 2, 512], BF16)
        grow = [
            singles.tile([1, BL], F32R, tag=f"grow{h}", name=f"grow{h}")
            for h in range(QH)
        ]  # gate rows per head
        rows = [
            singles.tile([1, BL], F32R, tag=f"row{i}", name=f"row{i}")
            for i in range(3)
        ]  # q0, q1, k rms rows -> reciprocals in place
        cache_tiles = {}
        vnew = {}

        def emit_prefetch():
            nc.scalar.dma_start(out=wo_sb, in_=wo)
            for b in range(B):
                ck_sb = singles.tile([128, CACHE], BF16, tag=f"ck{b}", name=f"ck{b}")
                nc.scalar.dma_start(out=ck_sb, in_=ckt[b])
                cv_sb = singles.tile([128, NSC, HD], BF16, tag=f"cv{b}", name=f"cv{b}")
                nc.scalar.dma_start(out=cv_sb, in_=cv[b])
                cache_tiles[b] = (ck_sb, cv_sb)
                vnew[b] = singles.tile(
                    [128, NSC, HD], BF16, tag=f"vn{b}", name=f"vn{b}"
                )

        def apply_norm(lc, jc):
            """Reciprocal of one rms row chunk + column normalize.  Deferred
            one chunk and spread between proj matmul bursts so the PE never
            waits on it."""
            sl = slice(lc * 512, lc * 512 + 512)
            nc.vector.reciprocal(rows[jc][:, sl], rows[jc][:, sl])
            bc = psM.tile([128, 512], F32, tag="m", name=f"bc{jc}_{lc}")
            nc.tensor.matmul(bc, onesr, rows[jc][:, sl], start=True, stop=True)
            nc.vector.tensor_mul(qkvt[:, jc, sl], qkvt[:, jc, sl], bc)

        # ---- phase 1: projections -------------------------------------
        for lc in range(NLP):
            sl = slice(lc * 512, lc * 512 + 512)
            xtile = xtp.tile([128, ND, 512], BF16, tag="xt")
            if lc == 0:
                for kq in range(4):
                    nc.sync.dma_start(
                        out=xtile[:, kq * 4 : kq * 4 + 4, :],
                        in_=xt[:, 0, kq * 4 : kq * 4 + 4, :],
                    )
                emit_prefetch()
            else:
                nc.sync.dma_start(out=xtile, in_=xt[:, lc])
            # gates for this chunk: one [1, 512] row per head
            for h in range(QH):
                gps = psM.tile([1, 512], F32, tag="m", name=f"gps{lc}{h}")
                nc.tensor.matmul(
                    gps, wg_sb[:, h : h + 1], xtile[0:H, 0, :],
                    start=True, stop=True,
                )
                nc.scalar.activation(grow[h][:, sl], gps, AF.Sigmoid)
            if lc >= 1:
                apply_norm(lc - 1, 2)  # k first: unblocks attention earliest
            for pi, pair in enumerate(((3, 2), (0, 1))):  # v,k first
                pp = psBig.tile([128, 2, 512], F32, tag="big", name=f"pp{lc}_{pi}")
                for j, jc in enumerate(pair):
                    for kk in range(ND):
                        nc.tensor.matmul(
                            pp[:, j, :],
                            wqkv_sb[:, kk, jc * 128 : jc * 128 + 128],
                            xtile[:, kk, :],
                            start=(kk == 0),
                            stop=(kk == ND - 1),
                        )
                for j, jc in enumerate(pair):
                    nc.vector.tensor_copy(qkvt[:, jc, sl], pp[:, j, :])
                    if jc < 3:  # q0, q1, k need sum over HD of the square
                        sq = sqp.tile([128, 512], BF16, tag="sq")
                        nc.scalar.activation(sq, pp[:, j, :], AF.Square)
                        ssq = psD.tile([1, 512], F32, tag="row", name=f"ssq{lc}{jc}")
                        nc.tensor.matmul(ssq, onesb, sq, start=True, stop=True)
                        # q: sqrt(ssq + HD*eps) (recip folds in 1/sqrt(HD));
                        # k: sqrt(ssq/HD + eps).
                        scale, bias = (1.0, bias_q) if jc < QH else (1.0 / HD, bias_k)
                        nc.scalar.activation(
                            rows[jc][:, sl], ssq, AF.Sqrt, bias=bias[:], scale=scale
                        )
                if lc >= 1 and pi == 0:
                    apply_norm(lc - 1, 0)
            if lc >= 1:
                apply_norm(lc - 1, 1)

        # v transposes (norm-independent) overlap the last norm chain
        apply_norm(NLP - 1, 2)
        for b in range(B):
            boff = b * L
            for i in range(NSC):
                tp = psO.tile([128, 128], BF16, tag="ot", name=f"tp{b}_{i}")
                nc.tensor.transpose(
                    tp, qkvt[:, 3, boff + i * 128 : boff + i * 128 + 128], identb
                )
                nc.vector.tensor_copy(vnew[b][:, i, :], tp)
                if b == 0 and i == 3:
                    apply_norm(NLP - 1, 0)
                if b == 0 and i == 7:
                    apply_norm(NLP - 1, 1)

        # ---- phase 2 + 3 interleaved ----------------------------------
        pending_pe = []   # deferred den/bc matmul emitters (prev block)
        pending_p3 = []   # deferred phase-3 unit emitters

        def emit_block(b, h, lc2):
            boff = b * L
            off = boff + lc2 * 512
            ck_sb, cv_sb = cache_tiles[b]
            q = qkvt[:, h, off : off + 512]
            ot = psO.tile([128, 512], F32, tag="ot", name=f"ot{b}{h}{lc2}")
            acc = [
                accp.tile([128, 2, 512], BF16, tag=t, name=f"ac{t}{b}{h}{lc2}")
                for t in ("A", "B")
            ]
            sps = {}
            exs = {}

            def emit_sp(i):
                sp = psBig.tile([128, 2, 512], F32, tag="big", name=f"sp{i}")
                for j in range(2):
                    sc = 2 * i + j
                    if sc < NSC:
                        kT = ck_sb[:, sc * 128 : sc * 128 + 128]
                    else:
                        jj = boff + (sc - NSC) * 128
                        kT = qkvt[:, 2, jj : jj + 128]
                    nc.tensor.matmul(sp[:, j, :], kT, q, start=True, stop=True)
                sps[i] = sp

            emit_sp(0)
            emit_sp(1)
            for fn in pending_pe:
                fn()
            pending_pe.clear()
            for i in range(NP):
                ex = exp_.tile([128, 2, 512], BF16, tag="ex", name=f"ex{i}")
                nc.scalar.activation(ex, sps[i], AF.Exp)
                exs[i] = ex
                for j in range(2):
                    sc = 2 * i + j
                    vx = cv_sb[:, sc, :] if sc < NSC else vnew[b][:, sc - NSC, :]
                    nc.tensor.matmul(
                        ot, vx, ex[:, j, :],
                        start=(sc == 0), stop=(sc == NS - 1),
                    )
                if i + 2 < NP:
                    emit_sp(i + 2)
                tgt = acc[i % 2]
                if i < 2:
                    nc.vector.tensor_copy(tgt, ex)
                else:
                    nc.vector.tensor_add(tgt, tgt, ex)
            # raw attention out -> SBUF immediately (frees PSUM)
            oslice = otg[:, b, h, lc2, :]
            nc.vector.tensor_copy(oslice, ot)
            # denominator: fold the tree, then (deferred) one ones-matmul
            nc.vector.tensor_add(acc[0], acc[0], acc[1])
            nc.vector.tensor_add(acc[0][:, 0, :], acc[0][:, 0, :], acc[0][:, 1, :])

            def den_chain(b=b, h=h, lc2=lc2, acc0=acc[0]):
                den = psD.tile([1, 512], F32, tag="row", name=f"den{b}{h}{lc2}")
                nc.tensor.matmul(den, onesb, acc0[:, 0, :], start=True, stop=True)
                fden = fdnp.tile([1, 512], F32R, tag="fd", name=f"fd{b}{h}{lc2}")
                nc.vector.reciprocal(fden, den)
                gc = b * 2 + lc2
                nc.vector.tensor_mul(
                    fden, fden, grow[h][:, gc * 512 : gc * 512 + 512]
                )
                bc = psM.tile([128, 512], F32, tag="m", name=f"bcd{b}{h}{lc2}")
                nc.tensor.matmul(bc, onesr, fden, start=True, stop=True)
                nc.vector.tensor_mul(
                    otg[:, b, h, lc2, :], otg[:, b, h, lc2, :], bc
                )

            pending_pe.append(den_chain)

        def emit_p3(b, lc2):
            for li in range(4):
                row0 = b * L + lc2 * 512 + li * 128
                ysb = ysbp.tile([128, 4, 512], BF16, tag="ysb", name=f"y{b}{lc2}{li}")
                for mc in range(4):
                    yp = psO.tile([128, 512], F32, tag="ot", name=f"yp{li}{mc}")
                    for h in range(QH):
                        nc.tensor.matmul(
                            yp,
                            otg[:, b, h, lc2, li * 128 : li * 128 + 128],
                            wo_sb[:, h, mc * 512 : mc * 512 + 512],
                            start=(h == 0),
                            stop=(h == QH - 1),
                        )
                    if mc < 2:
                        nc.vector.tensor_copy(ysb[:, mc, :], yp)
                    else:
                        nc.scalar.copy(ysb[:, mc, :], yp)
                nc.sync.dma_start(
                    out=y[row0 : row0 + 128, :],
                    in_=ysb.rearrange("p a b -> p (a b)"),
                )

        for b in range(B):
            for lc2 in range(2):
                for h in range(QH):
                    emit_block(b, h, lc2)
                    if h == 0 and pending_p3:
                        emit_p3(*pending_p3.pop(0))
                pending_p3.append((b, lc2))
        for fn in pending_pe:
            fn()
        pending_pe.clear()
        for unit in pending_p3:
            emit_p3(*unit)

    nc.compile()
    return nc


def _get_nc():
    global _CACHED_NC
    if _CACHED_NC is None:
        _CACHED_NC = _build_core_program()
    return _CACHED_NC


def make_in_maps(x, w_q, w_k, w_v, w_out, w_gate, cache_k, cache_v):
    bf = ml_dtypes.bfloat16
    xT = np.ascontiguousarray(x.reshape(BL, D).T)                 # [D, BL]
    xt = np.ascontiguousarray(
        xT.reshape(ND, 128, NLP, 512).transpose(1, 2, 0, 3)
    ).astype(bf)                                                  # [128,4,16,512]
    identb = np.eye(128, dtype=np.float32).astype(bf)
    onesb = np.ones((128, 1), np.float32).astype(bf)
    onesr = np.ones((1, 128), np.float32)
    in_maps = []
    for c in range(NCORES):
        g = c // 2
        wq_c = w_q[c * JC : (c + 1) * JC]                      # [256, D]
        wk_c = w_k[g * HD : (g + 1) * HD]                      # [128, D]
        wv_c = w_v[g * HD : (g + 1) * HD]
        wqkv_c = np.concatenate([wq_c, wk_c, wv_c], axis=0).T  # [D, 512]
        wqkv_c = np.ascontiguousarray(
            wqkv_c.reshape(ND, 128, 4 * HD).transpose(1, 0, 2)
        ).astype(bf)                                           # [128, 16, 512]
        wo_c = w_out[:, c * JC : (c + 1) * JC].T               # [256, D]
        wo_c = np.ascontiguousarray(
            wo_c.reshape(QH, 128, D).transpose(1, 0, 2)
        ).astype(bf)                                           # [128, 2, 2048]
        wg_c = np.ascontiguousarray(w_gate[c * QH : (c + 1) * QH].T).astype(bf)
        ckt_c = np.ascontiguousarray(cache_k[:, g].transpose(0, 2, 1)).astype(bf)
        cv_c = np.ascontiguousarray(
            cache_v[:, g].reshape(B, NSC, 128, HD).transpose(0, 2, 1, 3)
        ).astype(bf)                                           # [B,128,8,128]
        in_maps.append(
            {
                "xt": xt,
                "wqkv": wqkv_c,
                "wo": wo_c,
                "wg": wg_c,
                "ckt": ckt_c,
                "cv": cv_c,
                "identb": identb,
                "onesb": onesb,
                "onesr": onesr,
            }
        )
    return in_maps


def kernel(x, w_q, w_k, w_v, w_out, w_gate, cache_k, cache_v, _run_kwargs=None):
    in_maps = make_in_maps(x, w_q, w_k, w_v, w_out, w_gate, cache_k, cache_v)
    nc = _get_nc()
    res = run_bass_kernel_spmd(
        nc, in_maps, core_ids=list(range(NCORES)), **(_run_kwargs or {})
    )
    acc = np.zeros((BL, D), dtype=np.float32)
    for c in range(NCORES):
        acc += res.results[c]["y"].astype(np.float32)
    out = acc.reshape(B, L, D)
    if _run_kwargs:
        kernel.last_results = res
    return out


# revision 18
# speedup vs baseline: 1.6282x; 1.2594x over previous
"""Gated GQA self-attention with KV cache, tensor-parallel over heads on 8
Trainium2 NeuronCores.

Reference computation (fp32):
    q = rms_norm((x @ w_q.T).reshape(B,L,H,HD))      # per-head rms over HD
    k = rms_norm((x @ w_k.T).reshape(B,L,HKV,HD))
    v = (x @ w_v.T).reshape(B,L,HKV,HD)
    k_t/v_t = concat(cache, new) over seq -> [B,HKV,S,HD]
    o = softmax(q @ k_t.T / sqrt(HD)) @ v_t          # full (non-causal)
    o *= sigmoid(x[..., :16] @ w_gate.T)             # per-head gate
    y = o.reshape(B,L,D) @ w_out.T

Sharding: core c owns q heads {2c, 2c+1} and kv group g=c//2.  Each core
computes its heads' attention plus the partial out-projection
y_c = o_c @ w_out[:, cols_c].T; the host sums the 8 partials.

v3 design notes:
  * bf16 matmul operands everywhere; fp32 only in PSUM and the small
    normalization rows (~6e-3 max-rel total, gate is 2e-2).
  * Attention is one flat software pipeline over all 64 (block, s-pair)
    steps: scores for pair g+2 issue before P@V of pair g, crossing block
    boundaries, so the PE never drains and the HAM clock stays at 2.4GHz.
  * exp on [128,2,512] double-chunks (amortizes ACT fixed cost); softmax
    denominator via a DVE add-tree + one ones-matmul per block.
  * gate/denominator factors: rank-1 PE broadcast multiplied into the raw
    attention output once per block; the two heads then share one PSUM
    accumulation in the out-projection.  All row reciprocals use the ~5x
    faster reciprocal_approx_fast (18-bit accurate).
  * Out-projection emitted in 8-matmul li-bursts at block boundaries
    (deferred ~a block from its unit) - spreads y DMA and PE load.
  * Startup DMAs split k-chunk-wise so the first matmul waits on ~128KB,
    not megabytes.
"""

from contextlib import ExitStack

import ml_dtypes
import numpy as np

import concourse.bass as bass  # noqa: F401
import concourse.tile as tile
from concourse import bacc, mybir
from concourse.bass_utils import run_bass_kernel_spmd

F32R = mybir.dt.float32r
F32 = mybir.dt.float32
BF16 = mybir.dt.bfloat16
AF = mybir.ActivationFunctionType

B, L, D = 2, 1024, 2048
H, HKV, HD = 16, 4, 128
CACHE = 1024
BL = B * L                  # 2048
S = CACHE + L               # 2048
NCORES = 8
QH = H // NCORES            # 2 q heads per core
JC = QH * HD                # 256 out-proj contraction cols per core
EPS = 1e-6
NLP = BL // 512             # 4 l-chunks in phase 1
ND = D // 128               # 16 contraction chunks
NS = S // 128               # 16 s-chunks per batch
NSC = CACHE // 128          # 8 cached s chunks
NP = NS // 2                # 8 s-chunk pairs per block

_CACHED_NC = None


def _build_core_program():
    nc = bacc.Bacc("TRN2", target_bir_lowering=False, debug=False)

    xt = nc.dram_tensor("xt", [128, NLP, ND, 512], BF16, kind="ExternalInput").ap()
    wqkv = nc.dram_tensor("wqkv", [128, ND, 4 * HD], BF16, kind="ExternalInput").ap()
    wo = nc.dram_tensor("wo", [128, QH, D], BF16, kind="ExternalInput").ap()
    wg = nc.dram_tensor("wg", [H, QH], BF16, kind="ExternalInput").ap()
    ckt = nc.dram_tensor("ckt", [B, HD, CACHE], BF16, kind="ExternalInput").ap()
    cv = nc.dram_tensor("cv", [B, 128, NSC, HD], BF16, kind="ExternalInput").ap()
    identb_in = nc.dram_tensor("identb", [128, 128], BF16, kind="ExternalInput").ap()
    onesb_in = nc.dram_tensor("onesb", [128, 1], BF16, kind="ExternalInput").ap()
    onesr_in = nc.dram_tensor("onesr", [1, 128], F32R, kind="ExternalInput").ap()
    y = nc.dram_tensor("y", [BL, D], BF16, kind="ExternalOutput").ap()

    with tile.TileContext(nc) as tc, ExitStack() as ctx:
        singles = ctx.enter_context(tc.tile_pool(name="singles", bufs=1))
        xtp = ctx.enter_context(tc.tile_pool(name="xtp", bufs=2))
        sqp = ctx.enter_context(tc.tile_pool(name="sqp", bufs=2))
        exp_ = ctx.enter_context(tc.tile_pool(name="exp", bufs=6))
        accp = ctx.enter_context(tc.tile_pool(name="accp", bufs=2))
        fdnp = ctx.enter_context(tc.tile_pool(name="fdnp", bufs=2))
        ysbp = ctx.enter_context(tc.tile_pool(name="ysbp", bufs=2))

        psBig = ctx.enter_context(tc.tile_pool(name="psBig", bufs=2, space="PSUM"))
        psO = ctx.enter_context(tc.tile_pool(name="psO", bufs=2, space="PSUM"))
        psD = ctx.enter_context(tc.tile_pool(name="psD", bufs=1, space="PSUM"))
        psM = ctx.enter_context(tc.tile_pool(name="psM", bufs=1, space="PSUM"))

        lowp = nc.allow_low_precision(reason="bf16/f32r rounding is intended")
        ctx.enter_context(lowp)

        identb = singles.tile([128, 128], BF16)
        nc.scalar.dma_start(out=identb, in_=identb_in)
        onesb = singles.tile([128, 1], BF16)
        nc.scalar.dma_start(out=onesb, in_=onesb_in)
        onesr = singles.tile([1, 128], F32R)
        nc.scalar.dma_start(out=onesr, in_=onesr_in)
        wg_sb = singles.tile([H, QH], BF16)
        nc.scalar.dma_start(out=wg_sb, in_=wg)

        bias_q = singles.tile([1, 1], F32)
        nc.vector.memset(bias_q, HD * EPS)
        bias_k = singles.tile([1, 1], F32)
        nc.vector.memset(bias_k, EPS)

        # wqkv: k-chunk-major fine splits so the first LDWEIGHTS waits ~128KB
        wqkv_sb = singles.tile([128, ND, 4 * HD], BF16)
        for ka, kb in ((0, 1), (1, 4), (4, 10), (10, 16)):
            nc.sync.dma_start(
                out=wqkv_sb[:, ka:kb, :], in_=wqkv[:, ka:kb, :]
            )
        wo_sb = singles.tile([128, QH, D], BF16)

        qkvt = singles.tile([128, 4, BL], BF16)       # jc: qh0, qh1, k, v
        otg = singles.tile([128, B, QH, 2, 512], BF16)
        glog = [
            singles.tile([1, BL], F32R, tag=f"glog{h}", name=f"glog{h}")
            for h in range(QH)
        ]
        grow = [
            singles.tile([1, BL], F32R, tag=f"grow{h}", name=f"grow{h}")
            for h in range(QH)
        ]
        rows = [
            singles.tile([1, BL], F32R, tag=f"row{i}", name=f"row{i}")
            for i in range(3)
        ]
        cache_tiles = {}
        vnew = {}

        def emit_prefetch():
            nc.scalar.dma_start(out=wo_sb, in_=wo)
            for b in range(B):
                ck_sb = singles.tile([128, CACHE], BF16, tag=f"ck{b}", name=f"ck{b}")
                nc.scalar.dma_start(out=ck_sb, in_=ckt[b])
                cv_sb = singles.tile([128, NSC, HD], BF16, tag=f"cv{b}", name=f"cv{b}")
                nc.scalar.dma_start(out=cv_sb, in_=cv[b])
                cache_tiles[b] = (ck_sb, cv_sb)
                vnew[b] = singles.tile(
                    [128, NSC, HD], BF16, tag=f"vn{b}", name=f"vn{b}"
                )

        from concourse.dve_ops import (
            RECIP_APPROX_FAST_CONSTS as _RC,
            RECIPROCAL_APPROX_FAST as _RF,
        )

        def recip_fast(out_f32r, in_f32):
            """~18-bit 1/x on DVE, writing an f32r-typed row (the wrapper
            insists on fp32 both sides; f32r shares the fp32 bit layout)."""
            nc.vector._custom_dve(
                _RF, out=out_f32r, in0=in_f32,
                s0=_RC["s0"], s1=_RC["s1"], imm2=_RC["imm2"],
            )

        pe_defer = []  # (key, fn): deferred small PE ops, flushed between groups

        def flush_one_defer():
            if pe_defer:
                pe_defer.pop(0)[1]()

        def flush_key(key):
            """Run a specific deferred op now (prerequisite ordering)."""
            for idx, (k, fn) in enumerate(pe_defer):
                if k == key:
                    pe_defer.pop(idx)
                    fn()
                    return

        def apply_norm(lc, jc):
            """reciprocal of one rms row chunk + column normalize (deferred)."""
            flush_key(("ssq", lc, jc))  # the sqrt must be emitted before us
            sl = slice(lc * 512, lc * 512 + 512)
            recip_fast(rows[jc][:, sl], rows[jc][:, sl].bitcast(F32))

            def bc_mul(jc=jc, sl=sl, lc=lc):
                bc = psM.tile([128, 512], F32, tag="m", name=f"bc{jc}_{lc}")
                nc.tensor.matmul(bc, onesr, rows[jc][:, sl], start=True, stop=True)
                nc.vector.tensor_mul(qkvt[:, jc, sl], qkvt[:, jc, sl], bc)

            pe_defer.append((("bc", lc, jc), bc_mul))

        # ---- phase 1: projections -------------------------------------
        for lc in range(NLP):
            sl = slice(lc * 512, lc * 512 + 512)
            xtile = xtp.tile([128, ND, 512], BF16, tag="xt")
            if lc == 0:
                for ka, kb in ((0, 1), (1, 4), (4, 10), (10, 16)):
                    nc.sync.dma_start(
                        out=xtile[:, ka:kb, :], in_=xt[:, 0, ka:kb, :]
                    )
                emit_prefetch()
            else:
                nc.sync.dma_start(out=xtile, in_=xt[:, lc])
            # gate logit rows for this chunk
            for h in range(QH):
                gps = psM.tile([1, 512], F32, tag="m", name=f"gps{lc}{h}")
                nc.tensor.matmul(
                    gps, wg_sb[:, h : h + 1], xtile[0:H, 0, :],
                    start=True, stop=True,
                )
                nc.vector.tensor_copy(glog[h][:, sl], gps)
            if lc >= 1:
                apply_norm(lc - 1, 2)  # k first: unblocks attention earliest
            for pi, pair in enumerate(((3, 2), (0, 1))):  # v,k first
                pp = psBig.tile([128, 2, 512], F32, tag="big", name=f"pp{lc}_{pi}")
                for j, jc in enumerate(pair):
                    for kk in range(ND):
                        nc.tensor.matmul(
                            pp[:, j, :],
                            wqkv_sb[:, kk, jc * 128 : jc * 128 + 128],
                            xtile[:, kk, :],
                            start=(kk == 0),
                            stop=(kk == ND - 1),
                        )
                    flush_one_defer()
                for j, jc in enumerate(pair):
                    if jc < QH:
                        nc.scalar.copy(qkvt[:, jc, sl], pp[:, j, :])
                    else:
                        nc.vector.tensor_copy(qkvt[:, jc, sl], pp[:, j, :])
                    if jc < 3:  # q0, q1, k: rms stats
                        sq = sqp.tile([128, 512], BF16, tag="sq")
                        nc.vector.tensor_mul(sq, qkvt[:, jc, sl], qkvt[:, jc, sl])

                        def ssq_mm(jc=jc, sl=sl, sq=sq, lc=lc):
                            ssq = psD.tile(
                                [1, 512], F32, tag="row", name=f"ssq{lc}{jc}"
                            )
                            nc.tensor.matmul(ssq, onesb, sq, start=True, stop=True)
                            scale, bias = (
                                (1.0, bias_q) if jc < QH else (1.0 / HD, bias_k)
                            )
                            nc.scalar.activation(
                                rows[jc][:, sl], ssq, AF.Sqrt,
                                bias=bias[:], scale=scale,
                            )

                        pe_defer.append((("ssq", lc, jc), ssq_mm))
                    flush_one_defer()
                if lc >= 1 and pi == 0:
                    apply_norm(lc - 1, 0)
            if lc >= 1:
                apply_norm(lc - 1, 1)

        while pe_defer:
            flush_one_defer()
        apply_norm(NLP - 1, 2)
        # v transposes overlap the tail norm chains
        for b in range(B):
            boff = b * L
            for i in range(NSC):
                tp = psO.tile([128, 128], BF16, tag="ot", name=f"tp{b}_{i}")
                nc.tensor.transpose(
                    tp, qkvt[:, 3, boff + i * 128 : boff + i * 128 + 128], identb
                )
                nc.vector.tensor_copy(vnew[b][:, i, :], tp)
                if b == 0 and i == 1:
                    apply_norm(NLP - 1, 0)
                if b == 0 and i == 3:
                    apply_norm(NLP - 1, 1)
                if b == 0 and i in (5, 7) or b == 1 and i in (1, 3):
                    flush_one_defer()
        while pe_defer:
            flush_one_defer()
        # batched gate sigmoids (keeps the ACT table on EXP afterwards)
        for h in range(QH):
            nc.scalar.activation(grow[h], glog[h], AF.Sigmoid)

        # ---- phase 2 + 3: flat pipeline -------------------------------
        blocks = [(b, lc2, h) for b in range(B) for lc2 in range(2) for h in range(QH)]
        NB = len(blocks)
        st = [dict() for _ in range(NB)]
        pend_den = []   # den-chain emitters, flushed one block later
        pend_p3 = []    # (b, lc2, li) out-proj bursts

        def q_of(bi):
            b, lc2, h = blocks[bi]
            off = b * L + lc2 * 512
            return qkvt[:, h, off : off + 512]

        def emit_sp(bi, i):
            b, lc2, h = blocks[bi]
            boff = b * L
            ck_sb, _ = cache_tiles[b]
            sp = psBig.tile([128, 2, 512], F32, tag="big", name=f"sp{bi}_{i}")
            for j in range(2):
                sc = 2 * i + j
                if sc < NSC:
                    kT = ck_sb[:, sc * 128 : sc * 128 + 128]
                else:
                    jj = boff + (sc - NSC) * 128
                    kT = qkvt[:, 2, jj : jj + 128]
                nc.tensor.matmul(sp[:, j, :], kT, q_of(bi), start=True, stop=True)
            st[bi].setdefault("sps", {})[i] = sp

        def block_end(bi):
            b, lc2, h = blocks[bi]
            s = st[bi]
            nc.vector.tensor_copy(otg[:, b, h, lc2, :], s["ot"])
            a0, a1 = s["acc"]
            nc.vector.tensor_add(a0, a0, a1)
            nc.vector.tensor_add(a0[:, 0, :], a0[:, 0, :], a0[:, 1, :])

            def den_chain(b=b, h=h, lc2=lc2, a0=a0):
                den = psD.tile([1, 512], F32, tag="row", name=f"den{b}{h}{lc2}")
                nc.tensor.matmul(den, onesb, a0[:, 0, :], start=True, stop=True)
                fden = fdnp.tile([1, 512], F32R, tag="fd", name=f"fd{b}{h}{lc2}")
                recip_fast(fden, den)
                gc = b * 2 + lc2
                nc.vector.tensor_mul(
                    fden, fden, grow[h][:, gc * 512 : gc * 512 + 512]
                )
                bc = psM.tile([128, 512], F32, tag="m", name=f"bcd{b}{h}{lc2}")
                nc.tensor.matmul(bc, onesr, fden, start=True, stop=True)
                nc.vector.tensor_mul(
                    otg[:, b, h, lc2, :], otg[:, b, h, lc2, :], bc
                )

            pend_den.append(den_chain)

        def emit_p3_burst(b, lc2, li):
            row0 = b * L + lc2 * 512 + li * 128
            ysb = ysbp.tile([128, 4, 512], BF16, tag="ysb", name=f"y{b}{lc2}{li}")
            for mc in range(4):
                yp = psO.tile([128, 512], F32, tag="ot", name=f"yp{li}{mc}")
                for h in range(QH):
                    nc.tensor.matmul(
                        yp,
                        otg[:, b, h, lc2, li * 128 : li * 128 + 128],
                        wo_sb[:, h, mc * 512 : mc * 512 + 512],
                        start=(h == 0),
                        stop=(h == QH - 1),
                    )
                if mc < 2:
                    nc.vector.tensor_copy(ysb[:, mc, :], yp)
                else:
                    nc.scalar.copy(ysb[:, mc, :], yp)
            nc.sync.dma_start(
                out=y[row0 : row0 + 128, :],
                in_=ysb.rearrange("p a b -> p (a b)"),
            )

        pairs = [(bi, i) for bi in range(NB) for i in range(NP)]
        emit_sp(*pairs[0])
        emit_sp(*pairs[1])
        for g, (bi, i) in enumerate(pairs):
            b, lc2, h = blocks[bi]
            s = st[bi]
            if i == 0:
                s["ot"] = psO.tile([128, 512], F32, tag="ot", name=f"ot{bi}")
                s["acc"] = [
                    accp.tile([128, 2, 512], BF16, tag=t, name=f"ac{t}{bi}")
                    for t in ("A", "B")
                ]
            ex = exp_.tile([128, 2, 512], BF16, tag="ex", name=f"ex{bi}_{i}")
            nc.scalar.activation(ex, s["sps"].pop(i), AF.Exp)
            for j in range(2):
                sc = 2 * i + j
                vx = (
                    cache_tiles[b][1][:, sc, :]
                    if sc < NSC
                    else vnew[b][:, sc - NSC, :]
                )
                nc.tensor.matmul(
                    s["ot"], vx, ex[:, j, :],
                    start=(sc == 0), stop=(sc == NS - 1),
                )
            if i == 1 and pend_den:
                pend_den.pop(0)()
            if g + 2 < len(pairs):
                emit_sp(*pairs[g + 2])
            tgt = s["acc"][i % 2]
            if i < 2:
                nc.vector.tensor_copy(tgt, ex)
            else:
                nc.vector.tensor_add(tgt, tgt, ex)
            if i == NP - 1:
                block_end(bi)
                if h == QH - 1:
                    pend_p3.extend((b, lc2, li) for li in range(4))
                if bi >= 2:
                    for _ in range(2):
                        if pend_p3:
                            emit_p3_burst(*pend_p3.pop(0))
        while pend_den:
            pend_den.pop(0)()
        while pend_p3:
            emit_p3_burst(*pend_p3.pop(0))

    nc.compile()
    return nc


def _get_nc():
    global _CACHED_NC
    if _CACHED_NC is None:
        _CACHED_NC = _build_core_program()
    return _CACHED_NC


def make_in_maps(x, w_q, w_k, w_v, w_out, w_gate, cache_k, cache_v):
    bf = ml_dtypes.bfloat16
    xT = np.ascontiguousarray(x.reshape(BL, D).T)                 # [D, BL]
    xt = np.ascontiguousarray(
        xT.reshape(ND, 128, NLP, 512).transpose(1, 2, 0, 3)
    ).astype(bf)                                                  # [128,4,16,512]
    identb = np.eye(128, dtype=np.float32).astype(bf)
    onesb = np.ones((128, 1), np.float32).astype(bf)
    onesr = np.ones((1, 128), np.float32)
    in_maps = []
    for c in range(NCORES):
        g = c // 2
        wq_c = w_q[c * JC : (c + 1) * JC]                      # [256, D]
        wk_c = w_k[g * HD : (g + 1) * HD]                      # [128, D]
        wv_c = w_v[g * HD : (g + 1) * HD]
        wqkv_c = np.concatenate([wq_c, wk_c, wv_c], axis=0).T  # [D, 512]
        wqkv_c = np.ascontiguousarray(
            wqkv_c.reshape(ND, 128, 4 * HD).transpose(1, 0, 2)
        ).astype(bf)                                           # [128, 16, 512]
        wo_c = w_out[:, c * JC : (c + 1) * JC].T               # [256, D]
        wo_c = np.ascontiguousarray(
            wo_c.reshape(QH, 128, D).transpose(1, 0, 2)
        ).astype(bf)                                           # [128, 2, 2048]
        wg_c = np.ascontiguousarray(w_gate[c * QH : (c + 1) * QH].T).astype(bf)
        ckt_c = np.ascontiguousarray(cache_k[:, g].transpose(0, 2, 1)).astype(bf)
        cv_c = np.ascontiguousarray(
            cache_v[:, g].reshape(B, NSC, 128, HD).transpose(0, 2, 1, 3)
        ).astype(bf)                                           # [B,128,8,128]
        in_maps.append(
            {
                "xt": xt,
                "wqkv": wqkv_c,
                "wo": wo_c,
                "wg": wg_c,
                "ckt": ckt_c,
                "cv": cv_c,
                "identb": identb,
                "onesb": onesb,
                "onesr": onesr,
            }
        )
    return in_maps


def kernel(x, w_q, w_k, w_v, w_out, w_gate, cache_k, cache_v, _run_kwargs=None):
    in_maps = make_in_maps(x, w_q, w_k, w_v, w_out, w_gate, cache_k, cache_v)
    nc = _get_nc()
    res = run_bass_kernel_spmd(
        nc, in_maps, core_ids=list(range(NCORES)), **(_run_kwargs or {})
    )
    acc = np.zeros((BL, D), dtype=np.float32)
    for c in range(NCORES):
        acc += res.results[c]["y"].astype(np.float32)
    out = acc.reshape(B, L, D)
    if _run_kwargs:
        kernel.last_results = res
    return out
